# revision 19
# baseline (speedup 1.0000x reference)
"""Ragged GQA attention block (QKV proj + RoPE + paged-KV attention + WO proj)
on 8 TRN2 NeuronCores, tensor-parallel over heads.

v2 design (vs baseline): host pre-transposes x (ships xT), so all
projections run weight-STATIONARY with xT as the moving operand and land
directly in transposed layout (QT/KT/VT) -- no PE transposes of x, no Q/K
transposes, no PSUM->SBUF staging copies. RoPE in transposed layout needs a
cross-partition pair swap; that is done with one extra PE matmul per tile:
B = P @ A where P is the fixed "rotate-half" permutation (lhsT = P^T
constant), after which RoPE is 3 lane-local DVE ops with pair-replicated
cos/sin tables:  rot = cs1 (.) A + cs2 (.) B.

The 1/sqrt(HD) score scale is folded into the exp (activation scale).
Cached K is shipped host-transposed+packed; cached V host-packed, so KV
assembly is pure DMA. All ragged offsets in this problem instance are
128-aligned (asserted), which makes every attention piece full-width.

WO accumulates in SBUF (f32) per AllGather batch so attention PSUM and WO
PSUM coexist in the 8-bank budget.
"""

import math
import numpy as np

H, KVH, HD = 32, 8, 128
HIDDEN = H * HD            # 4096
T = 1024
TOTAL_KV = 3072
ROPE_THETA = 10000.0
N_CORES = 8
QH_PER = H // N_CORES      # 4 q heads per core
D2 = HD // 2
SCALE = 1.0 / math.sqrt(HD)
NEG = -1.0e30
KCH = HIDDEN // 128        # 32 contraction chunks
NKV = TOTAL_KV // 128      # 24 kv blocks

from contextlib import ExitStack

import concourse.bacc as bacc
import concourse.mybir as mybir
import concourse.tile as tile
from concourse.masks import make_identity
from concourse.bass_utils import run_bass_kernel_spmd

dt = mybir.dt
BF = dt.bfloat16
F32 = dt.float32
NCONST = 2 * T + 128       # cs1 | cs2 | tri


def build_nc(seqstarts, kvstarts, cachestarts, start_pos):
    """Trace + compile the SPMD Bass program, specialized to the offsets."""
    seqstarts = [int(v) for v in seqstarts]
    kvstarts = [int(v) for v in kvstarts]
    cachestarts = [int(v) for v in cachestarts]
    start_pos = [int(v) for v in start_pos]
    NB = len(start_pos)
    assert len(seqstarts) == NB + 1 and len(kvstarts) == NB + 1
    assert seqstarts[-1] == T and kvstarts[-1] == TOTAL_KV
    CSP = sum(start_pos)
    for b in range(NB):
        assert kvstarts[b + 1] - kvstarts[b] == start_pos[b] + (
            seqstarts[b + 1] - seqstarts[b]
        )
        for v in (seqstarts[b], kvstarts[b], start_pos[b]):
            assert v % 128 == 0, "this kernel assumes 128-aligned ragged offsets"
        assert kvstarts[b + 1] - kvstarts[b] <= 2048

    def tok_req(t):
        for b in range(NB):
            if seqstarts[b] <= t < seqstarts[b + 1]:
                return b
        raise AssertionError

    nc = bacc.Bacc(
        "TRN2", target_bir_lowering=False, debug=False, num_devices=N_CORES
    )
    xT_d = nc.dram_tensor("xT", [KCH, 128, T], BF, kind="ExternalInput").ap()
    wqkv_d = nc.dram_tensor(
        "wqkv_c", [KCH, 128, 768], BF, kind="ExternalInput"
    ).ap()
    wo_d = nc.dram_tensor("wo_c", [HIDDEN, 512], BF, kind="ExternalInput").ap()
    ckT_d = nc.dram_tensor(
        "ckT_c", [128, max(CSP, 128)], BF, kind="ExternalInput"
    ).ap()
    cv_d = nc.dram_tensor(
        "cv_c", [max(CSP, 128), 128], BF, kind="ExternalInput"
    ).ap()
    consts_d = nc.dram_tensor(
        "consts", [128, NCONST], F32, kind="ExternalInput"
    ).ap()
    ptc_d = nc.dram_tensor("ptc", [128, 128], BF, kind="ExternalInput").ap()
    outT_d = nc.dram_tensor("outT", [512, T], F32, kind="ExternalOutput").ap()

    warm_out = nc.dram_tensor("warm_out", [1, 64], BF, addr_space="Shared").ap()
    ag_out = [
        nc.dram_tensor(
            f"ag_out_{p}", [N_CORES * HD, 2 * T], BF, addr_space="Shared"
        ).ap()
        for p in range(QH_PER // 2)
    ]

    with tile.TileContext(nc) as tc:
        with ExitStack() as es:
            ec = es.enter_context
            cpool = ec(tc.tile_pool(name="consts", bufs=1))
            kt_pool = ec(tc.tile_pool(name="KT", bufs=1))
            v_pool = ec(tc.tile_pool(name="Vnat", bufs=1))
            qt_pool = ec(tc.tile_pool(name="QT", bufs=1))
            at_pool = ec(tc.tile_pool(name="attnT", bufs=1))
            rope_pool = ec(tc.tile_pool(name="rope", bufs=2))
            pr_pool = ec(tc.tile_pool(name="probs", bufs=2))
            pt_pool = ec(tc.tile_pool(name="ptsb", bufs=2))
            st_pool = ec(tc.tile_pool(name="stats", bufs=4))
            dramb = ec(tc.tile_pool(name="dramb", bufs=1, space="DRAM"))
            ps = None  # main PSUM pool, created after pass A's pool closes

            ident_bf = cpool.tile([128, 128], BF)
            make_identity(nc, ident_bf[:])

            # absorb the first-collective barrier/skew as early as possible
            warm_in = dramb.tile([1, 8], BF, name="warm_in")
            nc.sync.dma_start(warm_in[:], ident_bf[0:1, 0:8])
            nc.gpsimd.collective_compute(
                "AllGather",
                mybir.AluOpType.bypass,
                replica_groups=[list(range(N_CORES))],
                ins=[warm_in.opt()],
                outs=[warm_out],
            )
            consts = cpool.tile([128, NCONST], F32)
            ptc = cpool.tile([128, 128], BF)
            cs1 = consts[:, 0:T]
            cs2 = consts[:, T : 2 * T]
            tri = consts[:, 2 * T : 2 * T + 128]
            nc.sync.dma_start(consts[:], consts_d[:])
            nc.sync.dma_start(ptc[:], ptc_d[:])

            KT = kt_pool.tile([128, TOTAL_KV], BF)
            Vnat = v_pool.tile([128, NKV, HD], BF)
            QT = qt_pool.tile([128, QH_PER, T], BF)
            attnT = at_pool.tile([128, QH_PER, T], BF)

            # ---- cached K/V -> KT / Vnat (pure DMA; host packed) ----------
            off = 0
            for b in range(NB):
                sp, kb = start_pos[b], kvstarts[b]
                if sp:
                    nc.sync.dma_start(
                        KT[:, kb : kb + sp], ckT_d[:, off : off + sp]
                    )
                    nc.sync.dma_start(
                        Vnat[:, kb // 128 : (kb + sp) // 128, :],
                        cv_d[off : off + sp, :].rearrange(
                            "(j p) d -> p j d", p=128
                        ),
                    )
                    off += sp

            def rope(A_ps, half, segs, name, bpool=None, btag="qp"):
                """RoPE a [128,512] transposed psum tile.
                segs: list of (lo, hi, out_ap) with lo/hi relative cols in
                [0,512); out = cs1*A + cs2*(P@A) written per segment."""
                A_sb = rope_pool.tile(
                    [128, 512], BF, tag="asb", name=f"asb_{name}"
                )
                nc.vector.tensor_copy(A_sb[:], A_ps[:])
                B_ps = (bpool or ps).tile(
                    [128, 512], F32, tag=btag, name=f"qb_{name}",
                    bufs=(2 if bpool is None else None),
                )
                nc.tensor.matmul(B_ps[:], ptc[:], A_sb[:], start=True, stop=True)
                t1 = rope_pool.tile([128, 512], BF, tag="t1", name=f"t1_{name}")
                t2 = rope_pool.tile([128, 512], BF, tag="t2", name=f"t2_{name}")
                c0 = half * 512
                nc.vector.tensor_mul(t1[:], cs1[:, c0 : c0 + 512], A_ps[:])
                nc.vector.tensor_mul(t2[:], cs2[:, c0 : c0 + 512], B_ps[:])
                for lo, hi, out_ap in segs:
                    nc.vector.tensor_add(out_ap, t1[:, lo:hi], t2[:, lo:hi])

            def half_segs(half, dest_of_seg):
                """split this half's token range at request boundaries;
                dest_of_seg(b, tok_lo, tok_hi) -> output AP."""
                t0, t1_ = half * 512, half * 512 + 512
                segs = []
                cur = t0
                while cur < t1_:
                    b = tok_req(cur)
                    seg = min(t1_, seqstarts[b + 1])
                    segs.append((cur - t0, seg - t0, dest_of_seg(b, cur, seg)))
                    cur = seg
                return segs

            with ExitStack() as esx:
                xw_pool = esx.enter_context(tc.tile_pool(name="xw", bufs=1))
                xT = xw_pool.tile([128, KCH, T], BF)
                wq = xw_pool.tile([128, KCH, 768], BF)
                for k in range(KCH):
                    nc.sync.dma_start(xT[:, k, :], xT_d[k])
                    nc.sync.dma_start(wq[:, k, 0:384], wqkv_d[k, :, 0:384])
                for k in range(KCH):
                    nc.sync.dma_start(wq[:, k, 384:768], wqkv_d[k, :, 384:768])

                def kdest(b, lo, hi):
                    d = kvstarts[b] + start_pos[b] + (lo - seqstarts[b])
                    return KT[:, d : d + (hi - lo)]

                # ---- pass A: chunk-outer K + V + Q0 projection ------------
                # (PE consumes each chunk as its DMA lands: dense from t~0)
                VT_sb = rope_pool.tile([128, T], BF, tag="vt", name="vtsb")
                with tc.tile_pool(name="psA", bufs=1, space="PSUM") as psA:
                    nms = ["k0", "k1", "v0", "v1", "q0", "q1"]
                    acc = {
                        nm: psA.tile([128, 512], F32, tag=nm, name=f"pa_{nm}")
                        for nm in nms
                    }
                    for k in range(KCH):
                        for ci, base in ((0, 0), (1, 128), (2, 256)):
                            for half in range(2):
                                nc.tensor.matmul(
                                    acc[nms[ci * 2 + half]][:],
                                    wq[:, k, base : base + 128],
                                    xT[:, k, half * 512 : half * 512 + 512],
                                    start=(k == 0),
                                    stop=(k == KCH - 1),
                                )
                    for half in range(2):
                        rope(
                            acc[nms[half]], half, half_segs(half, kdest),
                            f"k{half}", bpool=psA, btag="pb",
                        )
                        nc.vector.tensor_copy(
                            VT_sb[:, half * 512 : half * 512 + 512],
                            acc[nms[2 + half]][:],
                        )
                        rope(
                            acc[nms[4 + half]], half,
                            [(0, 512, QT[:, 0, half * 512 : half * 512 + 512])],
                            f"q0_{half}", bpool=psA, btag="pb",
                        )

                ps = ec(tc.tile_pool(name="mainps", bufs=1, space="PSUM"))
                for tb in range(T // 128):
                    b = tok_req(tb * 128)
                    jb = (
                        kvstarts[b] + start_pos[b] + tb * 128 - seqstarts[b]
                    ) // 128
                    vtp = ps.tile([128, 128], BF, tag="aps", name=f"vtp_{tb}")
                    nc.tensor.transpose(
                        vtp[:], VT_sb[:, tb * 128 : (tb + 1) * 128], ident_bf[:]
                    )
                    nc.vector.tensor_copy(Vnat[:, jb, :], vtp[:])

                # ---- per-head: Q proj (h>0) + attention + AllGather -------
                for h in range(QH_PER):
                    for half in range(2) if h > 0 else ():
                        A_q = ps.tile(
                            [128, 512], F32, tag="qa", name=f"aq_{h}_{half}"
                        )
                        for k in range(KCH):
                            nc.tensor.matmul(
                                A_q[:],
                                wq[:, k, 256 + h * 128 : 384 + h * 128],
                                xT[:, k, half * 512 : half * 512 + 512],
                                start=(k == 0),
                                stop=(k == KCH - 1),
                            )
                        rope(
                            A_q,
                            half,
                            [
                                (
                                    0,
                                    512,
                                    QT[:, h, half * 512 : half * 512 + 512],
                                )
                            ],
                            f"q{h}_{half}",
                        )

                    # -- attention, software-pipelined over q blocks --------
                    def attn_s1(b, q0):
                        """scores (512-col psum slices) + softmax -> probs"""
                        kb, sp = kvstarts[b], start_pos[b]
                        L = sp + q0 + 128
                        qs = seqstarts[b] + q0
                        qT = QT[:, h, qs : qs + 128]
                        mlo = L - 128
                        probs = pr_pool.tile(
                            [128, 1280], BF, tag="probs",
                            name=f"pr_{h}_{b}_{q0}",
                        )
                        rsums = []
                        for n0 in range(0, L, 512):
                            n = min(512, L - n0)
                            sc = ps.tile(
                                [128, 512], F32, tag="sc", bufs=3,
                                name=f"sc_{h}_{b}_{q0}_{n0}",
                            )
                            nc.tensor.matmul(
                                sc[:, 0:n],
                                qT,
                                KT[:, kb + n0 : kb + n0 + n],
                                start=True,
                                stop=True,
                            )
                            if n0 + n > mlo:
                                o = mlo - n0
                                nc.vector.tensor_add(
                                    sc[:, o:n], sc[:, o:n], tri[:, 0 : n - o]
                                )
                            rs = st_pool.tile(
                                [128, 1], F32, tag=f"rs{n0 // 512}",
                                name=f"rs_{h}_{b}_{q0}_{n0}",
                            )
                            nc.scalar.activation(
                                probs[:, n0 : n0 + n],
                                sc[:, 0:n],
                                mybir.ActivationFunctionType.Exp,
                                bias=0.0,
                                scale=SCALE,
                                accum_out=rs[:],
                            )
                            rsums.append(rs)
                        for rs in rsums[1:]:
                            nc.vector.tensor_add(
                                rsums[0][:], rsums[0][:], rs[:]
                            )
                        rinv = st_pool.tile(
                            [128, 1], F32, tag="rinv",
                            name=f"ri_{h}_{b}_{q0}",
                        )
                        nc.vector.reciprocal(rinv[:], rsums[0][:])
                        nc.vector.tensor_scalar_mul(
                            probs[:, 0:L], probs[:, 0:L], rinv[:]
                        )
                        return (probs, L, kb, qs)

                    def attn_s2(st):
                        """probs transpose + PV -> attnT"""
                        probs, L, kb, qs = st
                        npc = L // 128
                        pt = pt_pool.tile(
                            [128, 1280], BF, tag="pt", name=f"pt_{h}_{qs}"
                        )
                        for g0 in range(0, npc, 8):
                            gl = min(8, npc - g0)
                            ptp = ps.tile(
                                [128, 1024], BF, tag="qp", bufs=2,
                                name=f"ptp_{h}_{qs}_{g0}",
                            )
                            for u in range(gl):
                                nc.tensor.transpose(
                                    ptp[:, u * 128 : (u + 1) * 128],
                                    probs[
                                        :, (g0 + u) * 128 : (g0 + u + 1) * 128
                                    ],
                                    ident_bf[:],
                                )
                            nc.vector.tensor_copy(
                                pt[:, g0 * 128 : (g0 + gl) * 128],
                                ptp[:, 0 : gl * 128],
                            )
                        aps = ps.tile(
                            [128, 128], F32, tag="aps", name=f"aps_{h}_{qs}"
                        )
                        for pi in range(npc):
                            nc.tensor.matmul(
                                aps[:],
                                Vnat[:, kb // 128 + pi, :],
                                pt[:, pi * 128 : (pi + 1) * 128],
                                start=(pi == 0),
                                stop=(pi == npc - 1),
                            )
                        nc.vector.tensor_copy(
                            attnT[:, h, qs : qs + 128], aps[:]
                        )

                    prev = None
                    for b in range(NB):
                        for q0 in range(0, seqstarts[b + 1] - seqstarts[b], 128):
                            st_new = attn_s1(b, q0)
                            if prev is not None:
                                attn_s2(prev)
                            prev = st_new
                    attn_s2(prev)

                    if h % 2 == 1:
                        p = h // 2
                        agi = dramb.tile([128, 2 * T], BF, name=f"agi{p}")
                        nc.sync.dma_start(
                            agi[:],
                            attnT[:, 2 * p : 2 * p + 2, :].rearrange(
                                "p h t -> p (h t)"
                            ),
                        )
                        nc.gpsimd.collective_compute(
                            "AllGather",
                            mybir.AluOpType.bypass,
                            replica_groups=[list(range(N_CORES))],
                            ins=[agi.opt()],
                            outs=[ag_out[p][:]],
                        )

            # ---- WO (column shard), SBUF f32 accumulation -----------------
            with ExitStack() as es5:
                ec5 = es5.enter_context
                af_pool = ec5(tc.tile_pool(name="af", bufs=1))
                wos_pool = ec5(tc.tile_pool(name="wos", bufs=1))
                acc_pool = ec5(tc.tile_pool(name="acc", bufs=1))
                acc = acc_pool.tile([128, 4, 2, 512], F32)
                for p in range(QH_PER // 2):
                    afs = []
                    wss = []
                    for s in range(2 * N_CORES):
                        r, hh = s // 2, 2 * p + s % 2
                        g = 4 * r + hh
                        af = af_pool.tile(
                            [128, T], BF, tag=f"af{s}", name=f"af_{p}_{s}"
                        )
                        nc.sync.dma_start(
                            af[:],
                            ag_out[p][
                                r * 128 : (r + 1) * 128,
                                (s % 2) * T : (s % 2 + 1) * T,
                            ],
                        )
                        ws = wos_pool.tile(
                            [128, 512], BF, tag=f"ws{s}", name=f"ws_{p}_{s}"
                        )
                        nc.sync.dma_start(ws[:], wo_d[g * 128 : (g + 1) * 128, :])
                        afs.append(af)
                        wss.append(ws)
                    for ocb in range(4):
                        for tt in range(2):
                            pw = ps.tile(
                                [128, 512], F32, tag="wo",
                                name=f"pw_{p}_{ocb}_{tt}",
                            )
                            for s in range(2 * N_CORES):
                                nc.tensor.matmul(
                                    pw[:],
                                    wss[s][:, ocb * 128 : (ocb + 1) * 128],
                                    afs[s][:, tt * 512 : (tt + 1) * 512],
                                    start=(s == 0),
                                    stop=(s == 2 * N_CORES - 1),
                                )
                            if p == 0:
                                nc.vector.tensor_copy(
                                    acc[:, ocb, tt, :], pw[:]
                                )
                            else:
                                nc.vector.tensor_add(
                                    acc[:, ocb, tt, :], acc[:, ocb, tt, :],
                                    pw[:],
                                )
                                nc.sync.dma_start(
                                    outT_d[
                                        ocb * 128 : (ocb + 1) * 128,
                                        tt * 512 : (tt + 1) * 512,
                                    ],
                                    acc[:, ocb, tt, :],
                                )

    nc.compile()
    return nc


def make_inputs(x, wqkv, wo, kv_cache, seqstarts, kvstarts, cachestarts, start_pos):
    """Host-side sharding: per-core input maps (weights/acts cast to bf16)."""
    import ml_dtypes

    bf16 = ml_dtypes.bfloat16
    x = np.asarray(x, dtype=np.float32)
    wqkv = np.asarray(wqkv, dtype=np.float32).astype(bf16)
    wo = np.asarray(wo, dtype=np.float32).astype(bf16)
    kv_cache = np.asarray(kv_cache, dtype=np.float32).astype(bf16)
    seqstarts = np.asarray(seqstarts)
    start_pos = np.asarray(start_pos)
    cachestarts = np.asarray(cachestarts)
    NB = len(start_pos)

    # xT chunked [KCH, 128, T]
    xT = np.ascontiguousarray(
        x.astype(bf16).T.reshape(KCH, 128, T)
    )

    tok = np.arange(T)
    bq = np.clip(
        np.searchsorted(seqstarts, tok, side="right") - 1, 0, NB - 1
    )
    pos_q = tok - seqstarts[bq] + start_pos[bq]
    inv_freq = 1.0 / (ROPE_THETA ** (np.arange(D2, dtype=np.float64) / D2))
    ang = pos_q[:, None].astype(np.float64) * inv_freq  # [T, 64]
    cos = np.cos(ang).astype(np.float32)                # [T, 64]
    sin = np.sin(ang).astype(np.float32)
    cs1 = np.repeat(cos.T, 2, axis=0)                   # [128, T]
    cs2 = np.repeat(sin.T, 2, axis=0)
    tri = np.where(
        np.arange(128)[None, :] <= np.arange(128)[:, None], 0.0, NEG
    ).astype(np.float32)
    consts = np.ascontiguousarray(
        np.concatenate([cs1, cs2, tri], axis=1)
    )

    # lhsT = P^T for the rotate-pair permutation:
    # B = P @ A with B[2d] = -A[2d+1], B[2d+1] = A[2d]
    ptc = np.zeros((128, 128), dtype=np.float32)
    idx = np.arange(D2)
    ptc[2 * idx + 1, 2 * idx] = -1.0
    ptc[2 * idx, 2 * idx + 1] = 1.0
    ptc = ptc.astype(bf16)

    CSP = int(start_pos.sum())

    in_maps = []
    for c in range(N_CORES):
        kcol = wqkv[:, HIDDEN + c * HD : HIDDEN + (c + 1) * HD]
        vcol = wqkv[
            :, HIDDEN + KVH * HD + c * HD : HIDDEN + KVH * HD + (c + 1) * HD
        ]
        qcol = wqkv[:, 512 * c : 512 * (c + 1)]
        wqkv_c = np.ascontiguousarray(
            np.concatenate([kcol, vcol, qcol], axis=1).reshape(KCH, 128, 768)
        )
        wo_c = np.ascontiguousarray(wo[:, 512 * c : 512 * (c + 1)])
        ck = kv_cache[0, 0][:, c, :]  # [8192, HD]
        cvv = kv_cache[0, 1][:, c, :]
        cks = []
        cvs = []
        for b in range(NB):
            sp = int(start_pos[b])
            if sp:
                cs0 = int(cachestarts[b])
                cks.append(ck[cs0 : cs0 + sp])
                cvs.append(cvv[cs0 : cs0 + sp])
        if cks:
            ckp = np.concatenate(cks, axis=0)
            cvp = np.concatenate(cvs, axis=0)
        else:
            ckp = np.zeros((128, HD), dtype=bf16)
            cvp = np.zeros((128, HD), dtype=bf16)
        if ckp.shape[0] < 128:
            pad = np.zeros((128 - ckp.shape[0], HD), dtype=bf16)
            ckp = np.concatenate([ckp, pad], axis=0)
            cvp = np.concatenate([cvp, pad], axis=0)
        ckT_c = np.ascontiguousarray(ckp.T)
        cv_c = np.ascontiguousarray(cvp)
        in_maps.append(
            dict(
                xT=xT, wqkv_c=wqkv_c, wo_c=wo_c, ckT_c=ckT_c, cv_c=cv_c,
                consts=consts, ptc=ptc,
            )
        )
    return in_maps


_NC_CACHE = {}


def _get_nc(key, seqstarts, kvstarts, cachestarts, start_pos):
    if key not in _NC_CACHE:
        _NC_CACHE[key] = build_nc(seqstarts, kvstarts, cachestarts, start_pos)
    return _NC_CACHE[key]


def run(inputs, trace=False, tmpdir=None):
    """Build (cached), run on 8 cores, return (full_output, BassKernelResults)."""
    seqstarts = np.asarray(inputs["seqstarts"]).tolist()
    kvstarts = np.asarray(inputs["kvstarts"]).tolist()
    cachestarts = np.asarray(inputs["cachestarts"]).tolist()
    start_pos = np.asarray(inputs["start_pos"]).tolist()
    key = tuple(seqstarts) + tuple(kvstarts) + tuple(cachestarts) + tuple(start_pos)
    nc = _get_nc(key, seqstarts, kvstarts, cachestarts, start_pos)
    in_maps = make_inputs(
        inputs["x"], inputs["wqkv"], inputs["wo"], inputs["kv_cache"],
        seqstarts, kvstarts, cachestarts, start_pos,
    )
    kw = {}
    if trace:
        kw = dict(trace=True, tmpdir=tmpdir)
    res = run_bass_kernel_spmd(nc, in_maps, list(range(N_CORES)), **kw)
    out = np.empty((T, HIDDEN), dtype=np.float32)
    for c in range(N_CORES):
        out[:, 512 * c : 512 * (c + 1)] = res.results[c]["outT"].T
    return out, res


def kernel(**inputs) -> np.ndarray:
    out, _ = run(inputs)
    return out
